# revision 28
# baseline (speedup 1.0000x reference)
"""FMoELinear grouped GEMM on 8 Trainium2 NeuronCores (expert parallelism).

Strategy
--------
Tokens arrive pre-grouped contiguously by expert, and the per-expert counts
are host-visible routing metadata.  All routing therefore happens on the
host: the 64 expert segments are split into pieces (piece counts chosen by
a small deterministic local search that minimises padded tiles), the pieces
are sorted by size and dealt in octets onto the 8 cores so that slot g has
the same tile count K[g] on every core.  That makes one SPMD Bass program
valid for all 8 cores while keeping padding small and per-core weight
traffic at ~G matrices.

Per core the device kernel computes, slot by slot, chunk by chunk:
    out[tile] = sum_k xT[k,tile].T @ wT[k]          (psum accumulation)
with x shipped host-transposed in [128 in-feat partitions, tile, k, token]
layout (12 KiB contiguous per partition line per chunk -> few large DMA
descriptors), and the output written in [128, tiles, 512] layout
(partition = token-within-tile) which the host untransposes on gather.

Numerics: MODE selects the matmul path.
  "f32"   - native fp32 matmuls (4 cycles/row on PE)
  "f32r"  - hardware round-to-~13-bit fast fp32 (1 cycle/row), ~1e-4 rel err
  "bf16"  - x, w, out all bf16 (psum accumulation stays f32): halves DMA
            bytes vs f32/f32r at the same 1 cycle/row PE speed, ~5e-3 max
            rel err (tolerance is 2e-2)
"""
import sys
sys.path.insert(0, "/opt/trn_rl_repo")

import random as _random

import numpy as np
import ml_dtypes

import concourse.bass as bass
import concourse.tile as tile
from concourse import bacc, mybir
from concourse.bass_utils import run_bass_kernel_spmd

# If the environment requests NTFF tracing (BASS_TRACE=1) but this image's
# antenv lacks the axon profiling hook module, run_bass_kernel_spmd would
# crash on import. Register a null hook so it degrades to trace-skipped.
try:
    from antenv.axon_hooks import get_axon_ntff_profile_hook as _hook_probe  # noqa: F401
except ImportError:
    import types as _types

    import antenv as _antenv

    _mod = _types.ModuleType("antenv.axon_hooks")
    _mod.get_axon_ntff_profile_hook = lambda: None
    _mod.set_axon_ntff_profile_hook = lambda h: None
    sys.modules.setdefault("antenv.axon_hooks", _mod)
    _antenv.axon_hooks = sys.modules["antenv.axon_hooks"]

F32 = mybir.dt.float32
F32R = mybir.dt.float32r
BF16 = mybir.dt.bfloat16

NUM_EXPERT = 64
IN_FEAT = 512
OUT_FEAT = 512
P = 128
KT = IN_FEAT // P          # 4 contraction k-tiles
NCORES = 8
CX = 12                    # max token tiles per x-DMA chunk
XBUFS = 7
OBUFS = 3
WBUFS = 3
WARMUP_MM = 8              # dummy matmuls at program start to ramp PE pstate

MODE = "bf16"              # "f32" | "f32r" | "bf16"
TRACE = False              # set True (e.g. from test.py) to profile
LAST_RESULT = None         # BassKernelResults of the last run

_program_cache = {}
_sched_cache = {}


# ----------------------------------------------------------------- schedule
def _schedule(counts):
    """Split experts into pieces, deal rank-octets onto cores.

    Returns (K, slots): K[g] = tile count of slot g (same on all cores);
    slots[core][g] = (expert, row_start, nrows) with nrows <= K[g]*128.

    Piece counts per expert are chosen by a seeded local search minimising
    sum(K) (the padded per-core tile count, which both the PE time and the
    x/out DMA bytes scale with) plus a small penalty per slot (weight DMA).
    """
    counts = [int(c) for c in counts]
    ckey = tuple(counts)
    if ckey in _sched_cache:
        return _sched_cache[ckey]
    starts = np.concatenate([[0], np.cumsum(counts)]).astype(np.int64)
    tiles = [(c + P - 1) // P for c in counts]
    live = [e for e in range(NUM_EXPERT) if tiles[e] > 0]

    def multiset(n_e):
        pieces = []
        for e in live:
            n = n_e[e]
            base, rem = divmod(tiles[e], n)
            pieces += [base + 1] * rem + ([base] * (n - rem) if base else [])
        pieces.sort(reverse=True)
        return pieces

    def cost(n_e):
        p = multiset(n_e)
        G = (len(p) + NCORES - 1) // NCORES
        sumK = sum(p[g * NCORES] for g in range(G))
        # one extra slot costs ~0.5MB weight DMA vs ~0.26MB (and ~0.9us PE)
        # per extra padded tile: weigh slots at ~1 tile
        return sumK + 1.0 * G

    rnd = _random.Random(12345)
    best_n, best_c = None, None
    for s0 in (8, 10, 12, 14, 16, 18, 20):
        n_e = {e: max(1, min(tiles[e], round(tiles[e] / s0))) for e in live}
        cur = cost(n_e)
        for _ in range(8000):
            e = live[rnd.randrange(len(live))]
            d = rnd.choice((-1, 1))
            v = n_e[e] + d
            if v < 1 or v > tiles[e]:
                continue
            n_e[e] = v
            new = cost(n_e)
            if new <= cur:
                cur = new
            else:
                n_e[e] = v - d
        if best_c is None or cur < best_c:
            best_c, best_n = cur, dict(n_e)

    pieces = []  # (ntiles, expert, tile_lo, tile_hi)
    for e in live:
        n = best_n[e]
        base, rem = divmod(tiles[e], n)
        lo = 0
        for i in range(n):
            sz = base + (1 if i < rem else 0)
            if sz == 0:
                continue
            pieces.append((sz, e, lo, lo + sz))
            lo += sz
    G = (len(pieces) + NCORES - 1) // NCORES
    while len(pieces) < NCORES * G:
        pieces.append((0, 0, 0, 0))
    pieces.sort(key=lambda t: (-t[0], t[1], t[2]))

    K = []
    slots = [[] for _ in range(NCORES)]
    for g in range(G):
        octet = pieces[g * NCORES:(g + 1) * NCORES]
        K.append(octet[0][0])
        for i, (sz, e, tlo, thi) in enumerate(octet):
            r0 = starts[e] + tlo * P
            r1 = min(starts[e] + thi * P, starts[e] + counts[e])
            slots[i].append((e, int(r0), max(0, int(r1 - r0))))
    # drop trailing zero-size slots
    while K and K[-1] == 0:
        K.pop()
        for s in slots:
            s.pop()
    _sched_cache[ckey] = (K, slots)
    return K, slots


def _slot_chunks(kg, first, last):
    """Chunk widths for one slot.  Ramp up at program start (small first
    chunks let the PE start before a full 12-tile DMA lands) and ramp down
    at program end (small final chunks shrink the post-compute drain)."""
    widths, rem = [], kg
    if first:
        for w in (2, 3, 4):
            if rem > w:
                widths.append(w)
                rem -= w
    tail = []
    if last:
        for w in (2, 3):
            if rem > w:
                tail.insert(0, w)
                rem -= w
    while rem > 0:
        w = min(CX, rem)
        widths.append(w)
        rem -= w
    return widths + tail


# ------------------------------------------------------------ device program
def _build_program(K, mode, has_bias=True):
    G = len(K)
    T = sum(K)
    nc = bacc.Bacc(None)

    mmdt = {"f32r": F32R, "f32": F32, "bf16": BF16}[mode]
    odt = BF16 if mode == "bf16" else F32
    # x layout [P, tile, k, token-in-tile]: per partition line a chunk is
    # cw*KT*P contiguous elements -> one large DMA descriptor per partition.
    xt_d = nc.declare_dram_parameter("xt", [P, T, KT, P], mmdt, isOutput=False)
    wt_d = nc.declare_dram_parameter("wt", [G, P, KT, OUT_FEAT], mmdt, isOutput=False)
    if has_bias:
        b_d = nc.declare_dram_parameter("bias", [G, 1, OUT_FEAT], F32, isOutput=False)
    out_d = nc.declare_dram_parameter("out", [P, T, OUT_FEAT], odt, isOutput=True)

    with tile.TileContext(nc) as tc:
        with (
            tc.tile_pool(name="w", bufs=WBUFS) as wp,
            tc.tile_pool(name="x", bufs=XBUFS) as xp,
            tc.tile_pool(name="b", bufs=2) as bp,
            tc.tile_pool(name="o", bufs=OBUFS) as op,
            tc.tile_pool(name="wu", bufs=1) as wup,
            tc.tile_pool(name="ps", bufs=8, space=bass.MemorySpace.PSUM) as pp,
        ):
            # Dummy matmuls on a zeroed scratch tile: no DMA dependency, so
            # the PE starts immediately and its pstate ramp (0.65 -> 2.4 GHz
            # over ~3us of continuous activity) is mostly done by the time
            # the first real operands land.
            z_sb = wup.tile([P, OUT_FEAT], mmdt)
            nc.gpsimd.memzero(z_sb[:])
            for _ in range(WARMUP_MM):
                ps = pp.tile([P, OUT_FEAT], F32)
                nc.tensor.matmul(ps[:], z_sb[:, :P], z_sb[:],
                                 start=True, stop=True)

            # Queue plan (SDMA engines round-robin between queues with
            # work, so streams on distinct queues share bandwidth): the x
            # stream alternates sync/gpsimd so it holds a 2/3 arbitration
            # share; out runs alone on scalar; weights on gpsimd.  Startup
            # exceptions: w0 rides scalar (idle early, fast HWDGE start)
            # and slot 0's x chunks all ride sync so the critical first
            # tiles never wait on the slow-starting gpsimd SWDGE path.
            gt = 0   # global tile counter (engine round-robin)
            ci = 0   # global chunk counter (x DMA queue round-robin)
            off = 0
            for g in range(G):
                kg = K[g]
                w_sb = wp.tile([P, KT, OUT_FEAT], mmdt, tag="w")
                if g == 0:
                    wq = nc.scalar
                elif g == 1:
                    wq = nc.sync
                else:
                    wq = nc.gpsimd
                wq.dma_start(w_sb[:], wt_d[g])
                if has_bias:
                    b1_sb = bp.tile([1, OUT_FEAT], F32, tag="b1")
                    nc.gpsimd.dma_start(b1_sb[:], b_d[g])
                    b_sb = bp.tile([P, OUT_FEAT], F32, tag="b")
                    nc.gpsimd.partition_broadcast(b_sb[:], b1_sb[:])

                c0 = 0
                for cw in _slot_chunks(kg, g == 0, g == G - 1):
                    x_sb = xp.tile([P, CX, KT, P], mmdt, tag="x")
                    # the gpsimd SWDGE queue needs ~15us before it moves
                    # bytes at full rate: keep the first ~26 tiles of x on
                    # the sync HWDGE queue, then alternate for a 2/3 share
                    if off + c0 < 26:
                        xq = nc.sync
                    else:
                        xq = nc.gpsimd if ci % 2 == 0 else nc.sync
                        ci += 1
                    xq.dma_start(
                        x_sb[:, :cw, :, :], xt_d[:, off + c0:off + c0 + cw, :, :])
                    o_sb = op.tile([P, CX, OUT_FEAT], odt)
                    for t in range(cw):
                        ps = pp.tile([P, OUT_FEAT], F32)
                        for k in range(KT):
                            nc.tensor.matmul(
                                ps[:], x_sb[:, t, k, :], w_sb[:, k, :],
                                start=(k == 0), stop=(k == KT - 1))
                        # psum -> sbuf (+bias, +downcast).  Pool (gpsimd)
                        # cannot read PSUM, so alternate DVE and Act.
                        if has_bias:
                            nc.vector.tensor_add(o_sb[:, t, :], ps[:], b_sb[:])
                        elif gt % 2 == 0:
                            nc.vector.tensor_copy(o_sb[:, t, :], ps[:])
                        else:
                            nc.scalar.copy(o_sb[:, t, :], ps[:])
                        gt += 1
                    nc.scalar.dma_start(
                        out_d[:, off + c0:off + c0 + cw, :], o_sb[:, :cw, :])
                    c0 += cw
                off += kg
    nc.compile()
    return nc


# ------------------------------------------------------------------- kernel
def kernel(inp, fwd_expert_count, weight, bias):
    inp = np.asarray(inp, dtype=np.float32)
    weight = np.asarray(weight, dtype=np.float32)
    bias = np.asarray(bias, dtype=np.float32)
    counts = np.asarray(fwd_expert_count)

    K, slots = _schedule(counts)
    G, T = len(K), sum(K)
    off = np.concatenate([[0], np.cumsum(K)]).astype(np.int64)

    has_bias = bool(np.any(bias))
    key = (tuple(K), MODE, has_bias)
    if key not in _program_cache:
        _program_cache[key] = _build_program(K, MODE, has_bias)
    nc = _program_cache[key]

    xdt = ml_dtypes.bfloat16 if MODE == "bf16" else np.float32

    # per-expert transposed weights [P, KT, OUT]: wT[p, k, o] = weight[e][o, 128k+p]
    wT = {}
    for e in set(e for s in slots for (e, _, n) in s if n > 0):
        wT[e] = np.ascontiguousarray(
            weight[e].T.reshape(KT, P, OUT_FEAT).transpose(1, 0, 2)).astype(xdt)

    in_maps = []
    for core in range(NCORES):
        xt = np.zeros((P, T, KT, P), dtype=xdt)
        wt = np.zeros((G, P, KT, OUT_FEAT), dtype=xdt)
        brep = np.zeros((G, 1, OUT_FEAT), dtype=np.float32)

        for g, (e, r0, n) in enumerate(slots[core]):
            if n > 0:
                A = inp[r0:r0 + n].astype(xdt)  # [n, IN]
                o0 = int(off[g])
                nf = n // P
                if nf:
                    xt[:, o0:o0 + nf] = (
                        A[:nf * P].reshape(nf, P, KT, P).transpose(3, 0, 2, 1))
                rem = n - nf * P
                if rem:
                    xt[:, o0 + nf, :, :rem] = (
                        A[nf * P:].reshape(rem, KT, P).transpose(2, 1, 0))
                wt[g] = wT[e]
                brep[g, 0] = bias[e]
        m = {"xt": xt, "wt": wt}
        if has_bias:
            m["bias"] = brep
        in_maps.append(m)

    global LAST_RESULT
    res = run_bass_kernel_spmd(
        nc, in_maps, list(range(NCORES)),
        trace=TRACE, trace_cores=list(range(NCORES)) if TRACE else None,
        stitch_traces=False)
    LAST_RESULT = res

    out = np.empty((int(np.sum(np.asarray(counts, dtype=np.int64))), OUT_FEAT),
                   dtype=np.float32)
    for core in range(NCORES):
        arr = np.asarray(res.results[core]["out"])  # [P, T, OUT], odt
        if arr.dtype != np.float32:
            arr = arr.astype(np.float32)
        for g, (e, r0, n) in enumerate(slots[core]):
            if n > 0:
                o0 = int(off[g])
                kg = K[g]
                blk = arr[:, o0:o0 + kg, :].transpose(1, 0, 2).reshape(kg * P, OUT_FEAT)
                out[r0:r0 + n] = blk[:n]
    return out
